# revision 1
# baseline (speedup 1.0000x reference)
"""Trainium2 Bass kernel for nn_Cross_modal_ContrastiveLoss6.

Math: the reference loss only depends on per-class means of the two
modalities (every entry of the N x N distance matrix is determined by the
class pair), so the whole computation reduces to:

  1. raw per-class segment sums R[c,d], T[c,d]  (memory-bound: 64 MiB read)
  2. the three 128x128 class Gram matrices P1 = R R^T, P2 = T T^T, P3 = R T^T
  3. tiny 128x128 class-pair loss math with the class counts

Device strategy (8 cores, feature/d-sharded so no cross-core collective is
needed): core k takes columns [256k, 256k+256) of both modal tensors and
computes the full-N segment sums for its d-chunk with one-hot matmuls on
the PE.  fp32 matmuls run at 1/8 the bf16 rate on trn2, so the host splits
the fp32 data into bf16 (hi, lo) pairs -- exact to ~2^-17 relative, same
total DMA bytes -- and the one-hot matrix is precomputed on the host in
bf16 (0/1 exact).  Everything is packed host-side into the exact SBUF
layout ([128 partitions, free]) so each DMA is a flat contiguous copy.
The device returns the raw hi/lo segment sums; the host recombines them,
forms the three Grams, and does the count scaling + sqrt/relu/weighted
mean (<0.1% of the FLOPs) in float64.
"""

import numpy as np
import ml_dtypes

import concourse.bacc as bacc
import concourse.bass as bass
import concourse.mybir as mybir
from concourse.bass_utils import run_bass_kernel_spmd
from concourse.tile import TileContext

N = 4096
D = 2048
C = 128
MARGIN = 0.5
NCORES = 8
DCHUNK = D // NCORES          # 256 feature columns per core
P = 128                       # partitions / sample-block size
NB = N // P                   # 32 sample blocks
# Variable x-DMA chunking (in 128-sample blocks): small chunks at the head
# (PE can start after 128 KiB lands) and tail (short last-chunk latency),
# big chunks in the middle (dense PE bursts keep the HAM clock-gate warm).
CHUNKS = [1, 1, 2, 4, 8, 8, 4, 2, 1, 1]
NCHUNK = len(CHUNKS)
CHUNK_OFF = [sum(CHUNKS[:i]) for i in range(NCHUNK + 1)]  # block offsets

F32 = mybir.dt.float32
BF16 = mybir.dt.bfloat16
NPBF16 = ml_dtypes.bfloat16

_PROGRAM = None


def _build_program() -> bass.Bass:
    """Raw-bass program (no TileContext): 4 engine streams, 5 semaphores.

    sync ring:   oh chunks + x1 chunks (interleaved) -> out DMA at the end
    scalar ring: x2 chunks
    tensor:      2 accumulation groups of 32 bf16 [128x128]x[128x512] matmuls
    vector:      2 PSUM->SBUF copies of the finished sums
    """
    nc = bass.Bass()

    # All inputs are packed host-side as [128 partitions, free] where
    # partition p of sample-block b is sample b*128+p.
    # consts[:, 0:128] = iota row (iota[p, c] = c), consts[:, 128:160] = targets
    # packed as tgt[p, b] = targets[b*128 + p]; the one-hot operand for the
    # matmuls is generated on the otherwise-idle DVE, saving 1 MiB of DMA.
    consts_in = nc.declare_dram_parameter("consts", [P, C + NB], F32, isOutput=False)
    x1_in = nc.declare_dram_parameter("x1", [P, NB * 512], BF16, isOutput=False)
    x2_in = nc.declare_dram_parameter("x2", [P, NB * 512], BF16, isOutput=False)
    # sums[:, 0:256] = R segment sums (hi+lo recombined), [:, 256:512] = T
    sums_out = nc.declare_dram_parameter("sums", [P, 512], F32, isOutput=True)


    import contextlib

    with contextlib.ExitStack() as stack:
        oh_t = stack.enter_context(nc.sbuf_tensor([P, NB * C], BF16))
        consts_t = stack.enter_context(nc.sbuf_tensor([P, C + NB], F32))
        x1_t = stack.enter_context(nc.sbuf_tensor([P, NB * 512], BF16))
        x2_t = stack.enter_context(nc.sbuf_tensor([P, NB * 512], BF16))
        warm_t = stack.enter_context(nc.sbuf_tensor([P, 136], BF16))
        out_t = stack.enter_context(nc.sbuf_tensor([P, 512], F32))
        psum_r = stack.enter_context(nc.psum_tensor([P, 256], F32))
        psum_t = stack.enter_context(nc.psum_tensor([P, 256], F32))
        psum_warm = stack.enter_context(nc.psum_tensor([P, 8], F32))

        # One dedicated semaphore per input DMA: a wait >= 16 then proves
        # exactly that transfer landed, with no assumption about completion
        # order between DMAs on the same ring.
        def sem(name):
            return stack.enter_context(nc.semaphore(name))

        consts_sem = sem("consts_dma")
        oh_gen = sem("oh_gen")
        x1_sems = [sem(f"x1_dma_{j}") for j in range(NCHUNK)]
        x2_sems = [sem(f"x2_dma_{j}") for j in range(NCHUNK)]
        pe_done = sem("pe_done")
        vec_done = sem("vec_done")
        dma_out = sem("dma_out")

        # Raw-bass semaphores are NOT cleared by the framework preamble;
        # stale values from whatever ran on the core before would satisfy
        # our waits early.  Clear them, then fence with the NRT pseudo
        # barrier (safe while bass sems are still being cleared).
        all_sems = [consts_sem, oh_gen] + x1_sems + x2_sems + [pe_done, vec_done, dma_out]
        nums = sorted(h.num for h in all_sems)
        assert nums == list(range(nums[0], nums[0] + len(nums))), nums
        sem_range = range(nums[0], nums[-1] + 1)
        nc.gpsimd.dma_reset(sem_range)
        nc.gpsimd.sem_clear(sem_range)
        nc._nrt_pseudo_barrier()

        # no_gpsimd_drain: skip the ~5us GpSimd DGE drain at block exit; the
        # block-exit engine drains + barrier fence everything that remains.
        with nc.Block(no_gpsimd_drain=True) as block:

            # Both modals' early chunks go out interleaved on ring 1 so the
            # PE's first R and T groups unblock as early as possible; ring 2
            # carries the consts + the later chunks.  The output is split
            # across both rings so the two DRAM write receipts overlap.
            SPLIT = 5

            @block.sync
            def _(sync: bass.BassEngine):
                for j in range(SPLIT):
                    sl = slice(CHUNK_OFF[j] * 512, CHUNK_OFF[j + 1] * 512)
                    sync.dma_start(
                        out=x1_t[:, sl], in_=x1_in[:, sl]
                    ).then_inc(x1_sems[j], 16)
                    sync.dma_start(
                        out=x2_t[:, sl], in_=x2_in[:, sl]
                    ).then_inc(x2_sems[j], 16)
                sync.wait_ge(vec_done, 1)
                sync.dma_start(
                    out=sums_out[:, 0:256], in_=out_t[:, 0:256]
                ).then_inc(dma_out, 16)
                sync.wait_ge(dma_out, 32)

            @block.scalar
            def _(scalar: bass.BassEngine):
                scalar.dma_start(out=consts_t[:], in_=consts_in[:]).then_inc(
                    consts_sem, 16
                )
                for j in range(SPLIT, NCHUNK):
                    sl = slice(CHUNK_OFF[j] * 512, CHUNK_OFF[j + 1] * 512)
                    scalar.dma_start(
                        out=x1_t[:, sl], in_=x1_in[:, sl]
                    ).then_inc(x1_sems[j], 16)
                    scalar.dma_start(
                        out=x2_t[:, sl], in_=x2_in[:, sl]
                    ).then_inc(x2_sems[j], 16)
                scalar.wait_ge(vec_done, 1)
                scalar.dma_start(
                    out=sums_out[:, 256:512], in_=out_t[:, 256:512]
                ).then_inc(dma_out, 16)
                scalar.wait_ge(dma_out, 32)

            @block.tensor
            def _(tensor: bass.BassEngine):
                # Warm the PE HAM clock gate (~3.5us of junk matmuls on
                # uninitialized scratch) while the first DMA chunks land, so
                # the real matmuls run at 2.4 GHz from the start.  Each cold
                # bf16 [128x128] load + [128x512] matmul is ~0.4us.
                for _ in range(12):
                    nc.tensor.matmul(
                        psum_warm[:],
                        warm_t[:, 0:128],
                        warm_t[:, 128:136],
                        start=True,
                        stop=True,
                    )
                for j in range(NCHUNK):
                    tensor.wait_ge(oh_gen, j + 1)
                    tensor.wait_ge(x1_sems[j], 16)
                    for b in range(CHUNK_OFF[j], CHUNK_OFF[j + 1]):
                        for h in range(2):  # hi then lo, same PSUM region
                            nc.tensor.matmul(
                                psum_r[:],
                                oh_t[:, b * C : (b + 1) * C],
                                x1_t[:, b * 512 + h * 256 : b * 512 + (h + 1) * 256],
                                start=(b == 0 and h == 0),
                                stop=(b == NB - 1 and h == 1),
                            )
                    tensor.wait_ge(x2_sems[j], 16)
                    for b in range(CHUNK_OFF[j], CHUNK_OFF[j + 1]):
                        for h in range(2):
                            nc.tensor.matmul(
                                psum_t[:],
                                oh_t[:, b * C : (b + 1) * C],
                                x2_t[:, b * 512 + h * 256 : b * 512 + (h + 1) * 256],
                                start=(b == 0 and h == 0),
                                stop=(b == NB - 1 and h == 1),
                            )
                # drain makes sure the last matmuls' PSUM writes have landed
                # before the DVE reads them.
                tensor.drain().then_inc(pe_done, 1)

            @block.vector
            def _(vector: bass.BassEngine):
                # Generate the bf16 one-hot blocks on the DVE, one x-chunk's
                # worth at a time: oh[p, b*C + c] = (targets[b*128+p] == c).
                vector.wait_ge(consts_sem, 16)
                for j in range(NCHUNK):
                    for b in range(CHUNK_OFF[j], CHUNK_OFF[j + 1]):
                        op = nc.vector.tensor_scalar(
                            oh_t[:, b * C : (b + 1) * C],
                            consts_t[:, 0:C],
                            consts_t[:, C + b : C + b + 1],
                            None,
                            mybir.AluOpType.is_equal,
                        )
                        if b == CHUNK_OFF[j + 1] - 1:
                            op.then_inc(oh_gen, 1)
                vector.wait_ge(pe_done, 1)
                nc.vector.tensor_copy(out_t[:, 0:256], psum_r[:])
                nc.vector.tensor_copy(out_t[:, 256:512], psum_t[:])
                vector.drain().then_inc(vec_done, 1)

    return nc


def _get_program() -> bass.Bass:
    global _PROGRAM
    if _PROGRAM is None:
        _PROGRAM = _build_program()
    return _PROGRAM


def _pack_blocks(x):
    """[4096, W] -> [128, NB*W] with partition p, block b at cols [b*W,(b+1)*W)."""
    W = x.shape[1]
    return np.ascontiguousarray(
        x.reshape(NB, P, W).transpose(1, 0, 2).reshape(P, NB * W)
    )


def _make_in_maps(modal1, modal2, targets):
    x1 = np.asarray(modal1, dtype=np.float32)
    x2 = np.asarray(modal2, dtype=np.float32)
    targets = np.asarray(targets)

    # bf16 hi/lo split (exact to ~2^-17 relative)
    def hilo(x):
        hi = x.astype(NPBF16)
        lo = (x - hi.astype(np.float32)).astype(NPBF16)
        return hi, lo

    x1_hi, x1_lo = hilo(x1)
    x2_hi, x2_lo = hilo(x2)

    tgt_pb = targets.reshape(NB, P).T.astype(np.float32)  # [p, b] = targets[b*128+p]
    iota = np.tile(np.arange(C, dtype=np.float32), (P, 1))
    consts = np.ascontiguousarray(np.concatenate([iota, tgt_pb], axis=1))

    in_maps = []
    for k in range(NCORES):
        sl = slice(k * DCHUNK, (k + 1) * DCHUNK)

        def pack_modal(hi, lo):
            # [4096, 512] = hi | lo for this core's d-chunk
            hl = np.concatenate([hi[:, sl], lo[:, sl]], axis=1)
            return _pack_blocks(hl)

        in_maps.append(
            {
                "consts": consts,
                "x1": pack_modal(x1_hi, x1_lo),
                "x2": pack_modal(x2_hi, x2_lo),
            }
        )
    return in_maps


def _finish_on_host(sums_list, targets):
    """Recombine hi/lo sums, form class Grams, and do the class-pair loss."""
    P1 = np.zeros((C, C), np.float64)
    P2 = np.zeros((C, C), np.float64)
    P3 = np.zeros((C, C), np.float64)
    for s in sums_list:
        s = np.asarray(s, np.float64)
        R = s[:, 0:256]                      # [class, d-chunk]
        T = s[:, 256:512]
        P1 += R @ R.T
        P2 += T @ T.T
        P3 += R @ T.T

    n = np.bincount(targets, minlength=C).astype(np.float64)
    u = 1.0 / np.maximum(n, 1.0)

    S_CC = P1 + P2 + P3 + P3.T  # (R+T)(R+T)^T
    uu = np.outer(u, u)
    A1 = 0.5 * uu * (P1 + P3)    # meanR . ctr
    A2 = 0.5 * uu * (P2 + P3.T)  # meanT . ctr
    nR = u * u * np.diag(P1)
    nT = u * u * np.diag(P2)
    nCtr = 0.25 * u * u * np.diag(S_CC)

    W = np.outer(n, n)
    eye = np.eye(C)
    total = 0.0
    for A, nrm in ((A1, nR), (A2, nT)):
        sq = np.maximum(nrm[:, None] + nCtr[None, :] - 2.0 * A, 1e-12)
        d = np.sqrt(sq)
        dd = np.sqrt(d + 1e-10)
        term = eye * sq + (1.0 - eye) * np.maximum(MARGIN - dd, 0.0) ** 2
        total += (W * term).sum() / (float(N) * float(N))
    return np.asarray(total, dtype=np.float32)


def kernel(modal1_inputs, modal2_inputs, targets):
    nc = _get_program()
    in_maps = _make_in_maps(modal1_inputs, modal2_inputs, targets)
    res = run_bass_kernel_spmd(nc, in_maps, list(range(NCORES)))
    sums_list = [res.results[k]["sums"] for k in range(NCORES)]
    return _finish_on_host(sums_list, np.asarray(targets))



# revision 2
# speedup vs baseline: 1.6235x; 1.6235x over previous
"""Trainium2 Bass kernel for nn_Cross_modal_ContrastiveLoss6.

Math: the reference loss only depends on per-class means of the two
modalities (every entry of the N x N distance matrix is determined by the
class pair), so the whole computation reduces to:

  1. raw per-class segment sums R[c,d], T[c,d]  (memory-bound pass over x)
  2. the three 128x128 class Gram matrices P1 = R R^T, P2 = T T^T, P3 = R T^T
  3. tiny 128x128 class-pair loss math with the class counts

Device strategy (8 cores, feature/d-sharded so no cross-core collective is
needed): core k takes columns [256k, 256k+256) of both modal tensors and
computes the full-N segment sums for its d-chunk with one-hot matmuls on
the PE.  The x data is quantized host-side to fp8 e4m3 (measured end-to-end
loss rel-err ~6e-4, well inside the 2e-2 gate) which makes the HBM read
2.1 MiB per core.  Each 128-sample block contributes one [128x128] x
[128x512] fp8 matmul (cols = [x1 256 | x2 256]) accumulating into a single
PSUM bank; the one-hot stationary operands are generated on the otherwise
idle DVE from a bf16 iota row and the packed targets via a broadcast-AP
is_equal.  The host recombines: forms the three Grams and does the count
scaling + sqrt/relu/weighted mean (<0.1% of the FLOPs) in float64.
"""

import numpy as np
import ml_dtypes

import concourse.bass as bass
import concourse.mybir as mybir
from concourse.bass_utils import run_bass_kernel_spmd

N = 4096
D = 2048
C = 128
MARGIN = 0.5
NCORES = 8
DCHUNK = D // NCORES          # 256 feature columns per core
P = 128                       # partitions / sample-block size
NB = N // P                   # 32 sample blocks
BWB = 512                     # fp8 bytes per partition per block (x1 256 | x2 256)
# Chunks (in 128-sample blocks), issued alternately on the sync / scalar
# HWDGE queues: small at the head (PE can start as soon as 64 KiB lands)
# and tail (short last-chunk latency), big in the middle for rate.
CHUNKS = [1, 1, 2, 2, 4, 4, 6, 6, 3, 3]
NCHUNK = len(CHUNKS)
CHUNK_OFF = [sum(CHUNKS[:i]) for i in range(NCHUNK + 1)]

F32 = mybir.dt.float32
BF16 = mybir.dt.bfloat16
FP8 = mybir.dt.float8e4
NPBF16 = ml_dtypes.bfloat16
NPFP8 = ml_dtypes.float8_e4m3

_PROGRAM = None


def _build_program() -> bass.Bass:
    nc = bass.Bass()

    # consts[:, 0:128] = iota row (iota[p, c] = c), consts[:, 128:160] =
    # targets packed as tgt[p, b] = targets[b*128 + p], both bf16 (exact for
    # 0..127).  x[:, b*512:(b+1)*512] = fp8 [x1 | x2] for sample block b.
    consts_in = nc.declare_dram_parameter("consts", [P, C + NB], BF16, isOutput=False)
    x_in = nc.declare_dram_parameter("x", [P, NB * BWB], FP8, isOutput=False)
    # sums[:, 0:256] = R segment sums for this core's d-chunk, [:, 256:512] = T
    sums_out = nc.declare_dram_parameter("sums", [P, 512], F32, isOutput=True)

    import contextlib

    with contextlib.ExitStack() as stack:
        consts_t = stack.enter_context(nc.sbuf_tensor([P, C + NB], BF16))
        oh_t = stack.enter_context(nc.sbuf_tensor([P, NB * C], FP8))
        x_t = stack.enter_context(nc.sbuf_tensor([P, NB * BWB], FP8))
        warm_t = stack.enter_context(nc.sbuf_tensor([P, 136], FP8))
        out_t = stack.enter_context(nc.sbuf_tensor([P, 512], F32))
        psum_acc = stack.enter_context(nc.psum_tensor([P, 512], F32))
        psum_warm = stack.enter_context(nc.psum_tensor([P, 8], F32))

        def sem(name):
            return stack.enter_context(nc.semaphore(name))

        consts_sem = sem("consts_dma")
        oh_gen = sem("oh_gen")
        x_sems = [sem(f"x_dma_{j}") for j in range(NCHUNK)]
        pe_done = sem("pe_done")
        vec_done = sem("vec_done")
        dma_out = sem("dma_out")

        # Raw-bass semaphores are NOT cleared by the framework preamble;
        # clear them, then fence with the NRT pseudo barrier (safe while
        # bass sems are still being cleared).
        all_sems = [consts_sem, oh_gen] + x_sems + [pe_done, vec_done, dma_out]
        nums = sorted(h.num for h in all_sems)
        assert nums == list(range(nums[0], nums[0] + len(nums))), nums
        sem_range = range(nums[0], nums[-1] + 1)
        nc.gpsimd.dma_reset(sem_range)
        nc.gpsimd.sem_clear(sem_range)
        nc._nrt_pseudo_barrier()

        with nc.Block(no_gpsimd_drain=True) as block:

            @block.sync
            def _(sync: bass.BassEngine):
                sync.dma_start(out=consts_t[:], in_=consts_in[:]).then_inc(
                    consts_sem, 16
                )
                for j in range(0, NCHUNK, 2):
                    sl = slice(CHUNK_OFF[j] * BWB, CHUNK_OFF[j + 1] * BWB)
                    sync.dma_start(out=x_t[:, sl], in_=x_in[:, sl]).then_inc(
                        x_sems[j], 16
                    )
                sync.wait_ge(vec_done, 1)
                sync.dma_start(
                    out=sums_out[:, 0:256], in_=out_t[:, 0:256]
                ).then_inc(dma_out, 16)
                sync.wait_ge(dma_out, 32)

            @block.scalar
            def _(scalar: bass.BassEngine):
                for j in range(1, NCHUNK, 2):
                    sl = slice(CHUNK_OFF[j] * BWB, CHUNK_OFF[j + 1] * BWB)
                    scalar.dma_start(out=x_t[:, sl], in_=x_in[:, sl]).then_inc(
                        x_sems[j], 16
                    )
                scalar.wait_ge(vec_done, 1)
                scalar.dma_start(
                    out=sums_out[:, 256:512], in_=out_t[:, 256:512]
                ).then_inc(dma_out, 16)
                scalar.wait_ge(dma_out, 32)

            @block.tensor
            def _(tensor: bass.BassEngine):
                # Warm the PE HAM clock gate with junk matmuls while the
                # first DMA chunks land, so the real matmuls run at full
                # clock from the start.
                for _ in range(12):
                    nc.tensor.matmul(
                        psum_warm[:],
                        warm_t[:, 0:128],
                        warm_t[:, 128:136],
                        start=True,
                        stop=True,
                    )
                for j in range(NCHUNK):
                    tensor.wait_ge(oh_gen, j + 1)
                    tensor.wait_ge(x_sems[j], 16)
                    for b in range(CHUNK_OFF[j], CHUNK_OFF[j + 1]):
                        nc.tensor.matmul(
                            psum_acc[:],
                            oh_t[:, b * C : (b + 1) * C],
                            x_t[:, b * BWB : (b + 1) * BWB],
                            start=(b == 0),
                            stop=(b == NB - 1),
                        )
                tensor.drain().then_inc(pe_done, 1)

            @block.vector
            def _(vector: bass.BassEngine):
                # One-hot generation: oh[p, b*C + c] = (c == targets[b*128+p])
                # as fp8, one tensor_tensor per x-chunk via broadcast APs.
                vector.wait_ge(consts_sem, 16)
                iota_ap = consts_t[:, 0:C].unsqueeze(1)
                for j in range(NCHUNK):
                    b0, b1 = CHUNK_OFF[j], CHUNK_OFF[j + 1]
                    nb = b1 - b0
                    o3 = oh_t[:, b0 * C : b1 * C].rearrange("p (j c) -> p j c", j=nb)
                    io3 = iota_ap.broadcast_to([P, nb, C])
                    tg3 = (
                        consts_t[:, C + b0 : C + b1]
                        .unsqueeze(2)
                        .broadcast_to([P, nb, C])
                    )
                    nc.vector.tensor_tensor(
                        o3, io3, tg3, mybir.AluOpType.is_equal
                    ).then_inc(oh_gen, 1)
                vector.wait_ge(pe_done, 1)
                nc.vector.tensor_copy(out_t[:], psum_acc[:])
                vector.drain().then_inc(vec_done, 1)

    return nc


def _get_program() -> bass.Bass:
    global _PROGRAM
    if _PROGRAM is None:
        _PROGRAM = _build_program()
    return _PROGRAM


def _make_in_maps(modal1, modal2, targets):
    x1 = np.asarray(modal1, dtype=np.float32)
    x2 = np.asarray(modal2, dtype=np.float32)
    targets = np.asarray(targets)

    x1q = x1.astype(NPFP8)
    x2q = x2.astype(NPFP8)

    tgt_pb = targets.reshape(NB, P).T.astype(np.float32)  # [p, b] = targets[b*128+p]
    iota = np.tile(np.arange(C, dtype=np.float32), (P, 1))
    consts = np.ascontiguousarray(
        np.concatenate([iota, tgt_pb], axis=1).astype(NPBF16)
    )

    in_maps = []
    for k in range(NCORES):
        sl = slice(k * DCHUNK, (k + 1) * DCHUNK)
        # [NB, P, 512] -> [P, NB*512]: block b cols = [x1 d-chunk | x2 d-chunk]
        blk = np.concatenate(
            [x1q[:, sl].reshape(NB, P, DCHUNK), x2q[:, sl].reshape(NB, P, DCHUNK)],
            axis=2,
        )
        xpk = np.ascontiguousarray(blk.transpose(1, 0, 2).reshape(P, NB * BWB))
        in_maps.append({"consts": consts, "x": xpk})
    return in_maps


def _finish_on_host(sums_list, targets):
    """Recombine per-core segment sums, form class Grams, do class-pair loss."""
    P1 = np.zeros((C, C), np.float64)
    P2 = np.zeros((C, C), np.float64)
    P3 = np.zeros((C, C), np.float64)
    for s in sums_list:
        s = np.asarray(s, np.float64)
        R = s[:, 0:256]                      # [class, d-chunk]
        T = s[:, 256:512]
        P1 += R @ R.T
        P2 += T @ T.T
        P3 += R @ T.T

    n = np.bincount(targets, minlength=C).astype(np.float64)
    u = 1.0 / np.maximum(n, 1.0)

    S_CC = P1 + P2 + P3 + P3.T  # (R+T)(R+T)^T
    uu = np.outer(u, u)
    A1 = 0.5 * uu * (P1 + P3)    # meanR . ctr
    A2 = 0.5 * uu * (P2 + P3.T)  # meanT . ctr
    nR = u * u * np.diag(P1)
    nT = u * u * np.diag(P2)
    nCtr = 0.25 * u * u * np.diag(S_CC)

    W = np.outer(n, n)
    eye = np.eye(C)
    total = 0.0
    for A, nrm in ((A1, nR), (A2, nT)):
        sq = np.maximum(nrm[:, None] + nCtr[None, :] - 2.0 * A, 1e-12)
        d = np.sqrt(sq)
        dd = np.sqrt(d + 1e-10)
        term = eye * sq + (1.0 - eye) * np.maximum(MARGIN - dd, 0.0) ** 2
        total += (W * term).sum() / (float(N) * float(N))
    return np.asarray(total, dtype=np.float32)


def kernel(modal1_inputs, modal2_inputs, targets):
    nc = _get_program()
    in_maps = _make_in_maps(modal1_inputs, modal2_inputs, targets)
    res = run_bass_kernel_spmd(nc, in_maps, list(range(NCORES)))
    sums_list = [res.results[k]["sums"] for k in range(NCORES)]
    return _finish_on_host(sums_list, np.asarray(targets))


# revision 3
# speedup vs baseline: 1.7209x; 1.0600x over previous
"""Trainium2 Bass kernel for nn_Cross_modal_ContrastiveLoss6.

Math: the reference loss only depends on per-class means of the two
modalities (every entry of the N x N distance matrix is determined by the
class pair), so the whole computation reduces to:

  1. raw per-class segment sums R[c,d], T[c,d]  (memory-bound pass over x)
  2. the three 128x128 class Gram matrices P1 = R R^T, P2 = T T^T, P3 = R T^T
  3. tiny 128x128 class-pair loss math with the class counts

Device strategy (8 cores, feature/d-sharded so no cross-core collective is
needed): core k takes columns [256k, 256k+256) of both modal tensors and
computes the full-N segment sums for its d-chunk with one-hot matmuls on
the PE.  The x data is quantized host-side to fp8 e4m3 (measured end-to-end
loss rel-err ~6e-4, well inside the 2e-2 gate) which makes the HBM read
2.1 MiB per core.  Each 128-sample block contributes one [128x128] x
[128x512] fp8 matmul (cols = [x1 256 | x2 256]) accumulating into a single
PSUM bank; the one-hot stationary operands are generated on the otherwise
idle DVE from an int8 iota row and the packed targets via a broadcast-AP
is_equal.  The segment sums return as bf16 (adds <1e-5 to the loss error);
the host recombines: forms the three Grams and does the count scaling +
sqrt/relu/weighted mean (<0.1% of the FLOPs) in float64.
"""

import numpy as np
import ml_dtypes

import concourse.bass as bass
import concourse.mybir as mybir
from concourse.bass_utils import run_bass_kernel_spmd

N = 4096
D = 2048
C = 128
MARGIN = 0.5
NCORES = 8
DCHUNK = D // NCORES          # 256 feature columns per core
P = 128                       # partitions / sample-block size
NB = N // P                   # 32 sample blocks
BWB = 512                     # fp8 bytes per partition per block (x1 256 | x2 256)
# Chunks (in 128-sample blocks), issued alternately on the sync / scalar
# HWDGE queues: small at the head (PE can start as soon as 128 KiB lands)
# and tail (short last-chunk latency), big in the middle for rate.
CHUNKS = [2, 2, 4, 4, 6, 6, 4, 4]
NCHUNK = len(CHUNKS)
CHUNK_OFF = [sum(CHUNKS[:i]) for i in range(NCHUNK + 1)]
NWARM = 14                    # junk matmuls to ramp the PE clock

F32 = mybir.dt.float32
BF16 = mybir.dt.bfloat16
I8 = mybir.dt.int8
FP8 = mybir.dt.float8e4
NPBF16 = ml_dtypes.bfloat16
NPFP8 = ml_dtypes.float8_e4m3

_PROGRAM = None


def _build_program() -> bass.Bass:
    nc = bass.Bass()

    # consts[:, 0:128] = iota row (iota[p, c] = c), consts[:, 128:160] =
    # targets packed as tgt[p, b] = targets[b*128 + p], both int8.
    # x[:, b*512:(b+1)*512] = fp8 [x1 | x2] for sample block b.
    consts_in = nc.declare_dram_parameter("consts", [P, C + NB], I8, isOutput=False)
    x_in = nc.declare_dram_parameter("x", [P, NB * BWB], FP8, isOutput=False)
    # sums[:, 0:256] = R segment sums for this core's d-chunk, [:, 256:512] = T
    sums_out = nc.declare_dram_parameter("sums", [P, 512], BF16, isOutput=True)

    import contextlib

    with contextlib.ExitStack() as stack:
        consts_t = stack.enter_context(nc.sbuf_tensor([P, C + NB], I8))
        oh_t = stack.enter_context(nc.sbuf_tensor([P, NB * C], FP8))
        x_t = stack.enter_context(nc.sbuf_tensor([P, NB * BWB], FP8))
        warm_t = stack.enter_context(nc.sbuf_tensor([P, 136], FP8))
        out_t = stack.enter_context(nc.sbuf_tensor([P, 512], BF16))
        psum_acc = stack.enter_context(nc.psum_tensor([P, 512], F32))
        psum_warm = stack.enter_context(nc.psum_tensor([P, 8], F32))

        def sem(name):
            return stack.enter_context(nc.semaphore(name))

        consts_sem = sem("consts_dma")
        oh_gen = sem("oh_gen")
        x_sems = [sem(f"x_dma_{j}") for j in range(NCHUNK)]
        pe_done = sem("pe_done")
        vec_done = sem("vec_done")
        dma_out = sem("dma_out")

        # Raw-bass semaphores are NOT cleared by the framework preamble; zero
        # them on gpsimd before the Block.  The Block-entry rendezvous keeps
        # every other engine from touching them until the clear has run, so
        # no extra barrier (and no DGE drain) is needed.
        all_sems = [consts_sem, oh_gen] + x_sems + [pe_done, vec_done, dma_out]
        nums = sorted(h.num for h in all_sems)
        assert nums == list(range(nums[0], nums[0] + len(nums))), nums
        nc.gpsimd.sem_clear(range(nums[0], nums[-1] + 1))

        with nc.Block(no_gpsimd_drain=True) as block:

            @block.sync
            def _(sync: bass.BassEngine):
                for j in range(0, NCHUNK, 2):
                    sl = slice(CHUNK_OFF[j] * BWB, CHUNK_OFF[j + 1] * BWB)
                    sync.dma_start(out=x_t[:, sl], in_=x_in[:, sl]).then_inc(
                        x_sems[j], 16
                    )
                sync.wait_ge(vec_done, 1)
                sync.dma_start(
                    out=sums_out[:, 0:256], in_=out_t[:, 0:256]
                ).then_inc(dma_out, 16)
                sync.wait_ge(dma_out, 32)

            @block.scalar
            def _(scalar: bass.BassEngine):
                scalar.dma_start(out=consts_t[:], in_=consts_in[:]).then_inc(
                    consts_sem, 16
                )
                for j in range(1, NCHUNK, 2):
                    sl = slice(CHUNK_OFF[j] * BWB, CHUNK_OFF[j + 1] * BWB)
                    scalar.dma_start(out=x_t[:, sl], in_=x_in[:, sl]).then_inc(
                        x_sems[j], 16
                    )
                scalar.wait_ge(vec_done, 1)
                scalar.dma_start(
                    out=sums_out[:, 256:512], in_=out_t[:, 256:512]
                ).then_inc(dma_out, 16)
                scalar.wait_ge(dma_out, 32)

            @block.tensor
            def _(tensor: bass.BassEngine):
                # Ramp the PE clock (HAM gate) with junk matmuls while the
                # first DMA chunks are in flight.
                for _ in range(NWARM):
                    nc.tensor.matmul(
                        psum_warm[:],
                        warm_t[:, 0:128],
                        warm_t[:, 128:136],
                        start=True,
                        stop=True,
                    )
                for j in range(NCHUNK):
                    tensor.wait_ge(oh_gen, j + 1)
                    tensor.wait_ge(x_sems[j], 16)
                    for b in range(CHUNK_OFF[j], CHUNK_OFF[j + 1]):
                        nc.tensor.matmul(
                            psum_acc[:],
                            oh_t[:, b * C : (b + 1) * C],
                            x_t[:, b * BWB : (b + 1) * BWB],
                            start=(b == 0),
                            stop=(b == NB - 1),
                        )
                tensor.drain().then_inc(pe_done, 1)

            @block.vector
            def _(vector: bass.BassEngine):
                # One-hot generation: oh[p, b*C + c] = (c == targets[b*128+p])
                # as fp8, one tensor_tensor per x-chunk via broadcast APs.
                vector.wait_ge(consts_sem, 16)
                iota_ap = consts_t[:, 0:C].unsqueeze(1)
                for j in range(NCHUNK):
                    b0, b1 = CHUNK_OFF[j], CHUNK_OFF[j + 1]
                    nb = b1 - b0
                    o3 = oh_t[:, b0 * C : b1 * C].rearrange("p (j c) -> p j c", j=nb)
                    io3 = iota_ap.broadcast_to([P, nb, C])
                    tg3 = (
                        consts_t[:, C + b0 : C + b1]
                        .unsqueeze(2)
                        .broadcast_to([P, nb, C])
                    )
                    nc.vector.tensor_tensor(
                        o3, io3, tg3, mybir.AluOpType.is_equal
                    ).then_inc(oh_gen, 1)
                vector.wait_ge(pe_done, 1)
                nc.vector.tensor_copy(out_t[:], psum_acc[:]).then_inc(vec_done, 1)

    return nc


def _get_program() -> bass.Bass:
    global _PROGRAM
    if _PROGRAM is None:
        _PROGRAM = _build_program()
    return _PROGRAM


def _make_in_maps(modal1, modal2, targets):
    x1 = np.asarray(modal1, dtype=np.float32)
    x2 = np.asarray(modal2, dtype=np.float32)
    targets = np.asarray(targets)

    x1q = x1.astype(NPFP8)
    x2q = x2.astype(NPFP8)

    tgt_pb = targets.reshape(NB, P).T.astype(np.int8)  # [p, b] = targets[b*128+p]
    iota = np.tile(np.arange(C, dtype=np.int8), (P, 1))
    consts = np.ascontiguousarray(np.concatenate([iota, tgt_pb], axis=1))

    in_maps = []
    for k in range(NCORES):
        sl = slice(k * DCHUNK, (k + 1) * DCHUNK)
        # [NB, P, 512] -> [P, NB*512]: block b cols = [x1 d-chunk | x2 d-chunk]
        blk = np.concatenate(
            [x1q[:, sl].reshape(NB, P, DCHUNK), x2q[:, sl].reshape(NB, P, DCHUNK)],
            axis=2,
        )
        xpk = np.ascontiguousarray(blk.transpose(1, 0, 2).reshape(P, NB * BWB))
        in_maps.append({"consts": consts, "x": xpk})
    return in_maps


def _finish_on_host(sums_list, targets):
    """Recombine per-core segment sums, form class Grams, do class-pair loss."""
    P1 = np.zeros((C, C), np.float64)
    P2 = np.zeros((C, C), np.float64)
    P3 = np.zeros((C, C), np.float64)
    for s in sums_list:
        s = np.asarray(s, np.float64)
        R = s[:, 0:256]                      # [class, d-chunk]
        T = s[:, 256:512]
        P1 += R @ R.T
        P2 += T @ T.T
        P3 += R @ T.T

    n = np.bincount(targets, minlength=C).astype(np.float64)
    u = 1.0 / np.maximum(n, 1.0)

    S_CC = P1 + P2 + P3 + P3.T  # (R+T)(R+T)^T
    uu = np.outer(u, u)
    A1 = 0.5 * uu * (P1 + P3)    # meanR . ctr
    A2 = 0.5 * uu * (P2 + P3.T)  # meanT . ctr
    nR = u * u * np.diag(P1)
    nT = u * u * np.diag(P2)
    nCtr = 0.25 * u * u * np.diag(S_CC)

    W = np.outer(n, n)
    eye = np.eye(C)
    total = 0.0
    for A, nrm in ((A1, nR), (A2, nT)):
        sq = np.maximum(nrm[:, None] + nCtr[None, :] - 2.0 * A, 1e-12)
        d = np.sqrt(sq)
        dd = np.sqrt(d + 1e-10)
        term = eye * sq + (1.0 - eye) * np.maximum(MARGIN - dd, 0.0) ** 2
        total += (W * term).sum() / (float(N) * float(N))
    return np.asarray(total, dtype=np.float32)


def kernel(modal1_inputs, modal2_inputs, targets):
    nc = _get_program()
    in_maps = _make_in_maps(modal1_inputs, modal2_inputs, targets)
    res = run_bass_kernel_spmd(nc, in_maps, list(range(NCORES)))
    sums_list = [res.results[k]["sums"] for k in range(NCORES)]
    return _finish_on_host(sums_list, np.asarray(targets))


# revision 5
# speedup vs baseline: 1.8391x; 1.0687x over previous
"""Trainium2 Bass kernel for nn_Cross_modal_ContrastiveLoss6.

Math: the reference loss only depends on per-class means of the two
modalities (every entry of the N x N distance matrix is determined by the
class pair), so the whole computation reduces to:

  1. raw per-class segment sums R[c,d], T[c,d]  (memory-bound pass over x)
  2. the three 128x128 class Gram matrices P1 = R R^T, P2 = T T^T, P3 = R T^T
  3. tiny 128x128 class-pair loss math with the class counts

Device strategy (8 cores, feature/d-sharded so no cross-core collective is
needed): core k takes columns [256k, 256k+256) of both modal tensors and
computes the full-N segment sums for its d-chunk with one-hot matmuls on
the PE.  The x data is quantized host-side to fp8 e4m3 (measured end-to-end
loss rel-err ~6e-4, well inside the 2e-2 gate) which makes the HBM read
2.1 MiB per core.  Sample blocks are processed in PAIRS with fp8 DoubleRow
matmuls ([128, 2, 128] one-hot stationary x [128, 2, 512] moving, 2 MACs
per cell per cycle) so the PE keeps up with the DMA stream even while the
power manager caps it at half clock during heavy DMA.  One-hot stationaries
are generated on the otherwise idle DVE from an int8 iota row and packed
targets via a broadcast-AP is_equal.  The segment sums return as bf16; the
host forms the three Grams and does the count scaling + sqrt/relu/weighted
mean (<0.1% of the FLOPs) in float64.
"""

import numpy as np
import ml_dtypes

import concourse.bass as bass
import concourse.mybir as mybir
from concourse.bass_utils import run_bass_kernel_spmd

N = 4096
D = 2048
C = 128
MARGIN = 0.5
NCORES = 8
DCHUNK = D // NCORES          # 256 feature columns per core
P = 128                       # partitions / sample-block size
NB = N // P                   # 32 sample blocks
BWB = 512                     # fp8 bytes per partition per block (x1 256 | x2 256)
# Chunks (in 128-sample blocks; all even so DoubleRow pairs never straddle),
# alternating sync/scalar HWDGE queues.  Entries on one queue stream
# back-to-back; only the completion (semaphore) of each entry lags its last
# byte by ~1.4us, so: small head chunks (fast pipeline start), big middle
# (few issue instructions, big descriptor lines), small tail (the last
# receipt gates only one pair of matmuls).
CHUNKS = [2, 2, 4, 4, 8, 8, 2, 2]
NCHUNK = len(CHUNKS)
CHUNK_OFF = [sum(CHUNKS[:i]) for i in range(NCHUNK + 1)]
NWARM = 10                    # junk matmuls to ramp the PE clock

F32 = mybir.dt.float32
BF16 = mybir.dt.bfloat16
I8 = mybir.dt.int8
FP8 = mybir.dt.float8e4
NPBF16 = ml_dtypes.bfloat16
NPFP8 = ml_dtypes.float8_e4m3

_PROGRAM = None


def _build_program() -> bass.Bass:
    nc = bass.Bass()

    # consts[:, 0:128] = iota row (iota[p, c] = c), consts[:, 128:160] =
    # targets packed as tgt[p, b] = targets[b*128 + p], both int8.
    # x[:, b*512:(b+1)*512] = fp8 [x1 | x2] for sample block b.
    consts_in = nc.declare_dram_parameter("consts", [P, C + NB], I8, isOutput=False)
    x_in = nc.declare_dram_parameter("x", [P, NB * BWB], FP8, isOutput=False)
    # sums[:, 0:256] = R segment sums for this core's d-chunk, [:, 256:512] = T
    sums_out = nc.declare_dram_parameter("sums", [P, 512], BF16, isOutput=True)

    import contextlib

    with contextlib.ExitStack() as stack:
        consts_t = stack.enter_context(nc.sbuf_tensor([P, C + NB], I8))
        oh_t = stack.enter_context(nc.sbuf_tensor([P, NB * C], FP8))
        x_t = stack.enter_context(nc.sbuf_tensor([P, NB * BWB], FP8))
        warm_t = stack.enter_context(nc.sbuf_tensor([P, 640], FP8))
        out_t = stack.enter_context(nc.sbuf_tensor([P, 512], BF16))
        psum_acc = stack.enter_context(nc.psum_tensor([P, 512], F32))
        psum_warm = stack.enter_context(nc.psum_tensor([P, 512], F32))

        def sem(name):
            return stack.enter_context(nc.semaphore(name))

        consts_sem = sem("consts_dma")
        oh_gen = sem("oh_gen")
        x_sems = [sem(f"x_dma_{j}") for j in range(NCHUNK)]
        pe_done = sem("pe_done")
        vec_done = sem("vec_done")
        dma_out = sem("dma_out")

        # Raw-bass semaphores are NOT cleared by the framework preamble; zero
        # them on gpsimd before the Block.  The Block-entry rendezvous keeps
        # every other engine from touching them until the clear has run.
        all_sems = [consts_sem, oh_gen] + x_sems + [pe_done, vec_done, dma_out]
        nums = sorted(h.num for h in all_sems)
        assert nums == list(range(nums[0], nums[0] + len(nums))), nums
        nc.gpsimd.sem_clear(range(nums[0], nums[-1] + 1))

        with nc.Block(no_gpsimd_drain=True) as block:

            @block.sync
            def _(sync: bass.BassEngine):
                sync.dma_start(out=consts_t[:], in_=consts_in[:]).then_inc(
                    consts_sem, 16
                )
                for j in range(0, NCHUNK, 2):
                    sl = slice(CHUNK_OFF[j] * BWB, CHUNK_OFF[j + 1] * BWB)
                    sync.dma_start(out=x_t[:, sl], in_=x_in[:, sl]).then_inc(
                        x_sems[j], 16
                    )
                sync.wait_ge(vec_done, 1)
                sync.dma_start(
                    out=sums_out[:, 0:256], in_=out_t[:, 0:256]
                ).then_inc(dma_out, 16)
                sync.wait_ge(dma_out, 32)

            @block.scalar
            def _(scalar: bass.BassEngine):
                for j in range(1, NCHUNK, 2):
                    sl = slice(CHUNK_OFF[j] * BWB, CHUNK_OFF[j + 1] * BWB)
                    scalar.dma_start(out=x_t[:, sl], in_=x_in[:, sl]).then_inc(
                        x_sems[j], 16
                    )
                scalar.wait_ge(vec_done, 1)
                scalar.dma_start(
                    out=sums_out[:, 256:512], in_=out_t[:, 256:512]
                ).then_inc(dma_out, 16)
                scalar.wait_ge(dma_out, 32)

            @block.tensor
            def _(tensor: bass.BassEngine):
                # Ramp the PE clock (HAM gate) with junk matmuls while the
                # first DMA chunks are in flight.
                for _ in range(NWARM):
                    nc.tensor.matmul(
                        psum_warm[:],
                        warm_t[:, 0:128],
                        warm_t[:, 128:640],
                        start=True,
                        stop=True,
                    )
                for j in range(NCHUNK):
                    tensor.wait_ge(oh_gen, j + 1)
                    tensor.wait_ge(x_sems[j], 16)
                    for pr in range(CHUNK_OFF[j] // 2, CHUNK_OFF[j + 1] // 2):
                        lhsT = oh_t[:, pr * 2 * C : (pr + 1) * 2 * C].rearrange(
                            "p (r c) -> p r c", r=2
                        )
                        rhs = x_t[:, pr * 2 * BWB : (pr + 1) * 2 * BWB].rearrange(
                            "p (r w) -> p r w", r=2
                        )
                        nc.tensor.matmul(
                            psum_acc[:],
                            lhsT,
                            rhs,
                            start=(pr == 0),
                            stop=(pr == NB // 2 - 1),
                            perf_mode=mybir.MatmulPerfMode.DoubleRow,
                        )
                tensor.drain().then_inc(pe_done, 1)

            @block.vector
            def _(vector: bass.BassEngine):
                # One-hot generation: oh[p, b*C + c] = (c == targets[b*128+p])
                # as fp8, one tensor_tensor per x-chunk via broadcast APs.
                vector.wait_ge(consts_sem, 16)
                iota_ap = consts_t[:, 0:C].unsqueeze(1)
                for j in range(NCHUNK):
                    b0, b1 = CHUNK_OFF[j], CHUNK_OFF[j + 1]
                    nb = b1 - b0
                    o3 = oh_t[:, b0 * C : b1 * C].rearrange("p (j c) -> p j c", j=nb)
                    io3 = iota_ap.broadcast_to([P, nb, C])
                    tg3 = (
                        consts_t[:, C + b0 : C + b1]
                        .unsqueeze(2)
                        .broadcast_to([P, nb, C])
                    )
                    nc.vector.tensor_tensor(
                        o3, io3, tg3, mybir.AluOpType.is_equal
                    ).then_inc(oh_gen, 1)
                vector.wait_ge(pe_done, 1)
                nc.vector.tensor_copy(out_t[:], psum_acc[:]).then_inc(vec_done, 1)

    return nc


def _get_program() -> bass.Bass:
    global _PROGRAM
    if _PROGRAM is None:
        _PROGRAM = _build_program()
    return _PROGRAM


def _make_in_maps(modal1, modal2, targets):
    x1 = np.asarray(modal1, dtype=np.float32)
    x2 = np.asarray(modal2, dtype=np.float32)
    targets = np.asarray(targets)

    x1q = x1.astype(NPFP8)
    x2q = x2.astype(NPFP8)

    tgt_pb = targets.reshape(NB, P).T.astype(np.int8)  # [p, b] = targets[b*128+p]
    iota = np.tile(np.arange(C, dtype=np.int8), (P, 1))
    consts = np.ascontiguousarray(np.concatenate([iota, tgt_pb], axis=1))

    in_maps = []
    for k in range(NCORES):
        sl = slice(k * DCHUNK, (k + 1) * DCHUNK)
        # [NB, P, 512] -> [P, NB*512]: block b cols = [x1 d-chunk | x2 d-chunk]
        blk = np.concatenate(
            [x1q[:, sl].reshape(NB, P, DCHUNK), x2q[:, sl].reshape(NB, P, DCHUNK)],
            axis=2,
        )
        xpk = np.ascontiguousarray(blk.transpose(1, 0, 2).reshape(P, NB * BWB))
        in_maps.append({"consts": consts, "x": xpk})
    return in_maps


def _finish_on_host(sums_list, targets):
    """Recombine per-core segment sums, form class Grams, do class-pair loss."""
    P1 = np.zeros((C, C), np.float64)
    P2 = np.zeros((C, C), np.float64)
    P3 = np.zeros((C, C), np.float64)
    for s in sums_list:
        s = np.asarray(s, np.float64)
        R = s[:, 0:256]                      # [class, d-chunk]
        T = s[:, 256:512]
        P1 += R @ R.T
        P2 += T @ T.T
        P3 += R @ T.T

    n = np.bincount(targets, minlength=C).astype(np.float64)
    u = 1.0 / np.maximum(n, 1.0)

    S_CC = P1 + P2 + P3 + P3.T  # (R+T)(R+T)^T
    uu = np.outer(u, u)
    A1 = 0.5 * uu * (P1 + P3)    # meanR . ctr
    A2 = 0.5 * uu * (P2 + P3.T)  # meanT . ctr
    nR = u * u * np.diag(P1)
    nT = u * u * np.diag(P2)
    nCtr = 0.25 * u * u * np.diag(S_CC)

    W = np.outer(n, n)
    eye = np.eye(C)
    total = 0.0
    for A, nrm in ((A1, nR), (A2, nT)):
        sq = np.maximum(nrm[:, None] + nCtr[None, :] - 2.0 * A, 1e-12)
        d = np.sqrt(sq)
        dd = np.sqrt(d + 1e-10)
        term = eye * sq + (1.0 - eye) * np.maximum(MARGIN - dd, 0.0) ** 2
        total += (W * term).sum() / (float(N) * float(N))
    return np.asarray(total, dtype=np.float32)


def kernel(modal1_inputs, modal2_inputs, targets):
    nc = _get_program()
    in_maps = _make_in_maps(modal1_inputs, modal2_inputs, targets)
    res = run_bass_kernel_spmd(nc, in_maps, list(range(NCORES)))
    sums_list = [res.results[k]["sums"] for k in range(NCORES)]
    return _finish_on_host(sums_list, np.asarray(targets))


# revision 7
# speedup vs baseline: 2.0066x; 1.0911x over previous
"""Trainium2 Bass kernel for nn_Cross_modal_ContrastiveLoss6.

Math: the reference loss only depends on per-class means of the two
modalities (every entry of the N x N distance matrix is determined by the
class pair), so the whole computation reduces to:

  1. raw per-class segment sums R[c,d], T[c,d]  (memory-bound pass over x)
  2. the three 128x128 class Gram matrices P1 = R R^T, P2 = T T^T, P3 = R T^T
  3. tiny 128x128 class-pair loss math with the class counts

Device strategy (8 cores, feature/d-sharded so no cross-core collective is
needed): core k takes columns [256k, 256k+256) of both modal tensors and
computes the full-N segment sums for its d-chunk with one-hot matmuls on
the PE.  The x data is quantized host-side to fp8 e4m3 (measured end-to-end
loss rel-err ~6e-4, well inside the 2e-2 gate) which makes the HBM read
2.1 MiB per core.  Sample blocks are processed in PAIRS with fp8 DoubleRow
matmuls ([128, 2, 128] one-hot stationary x [128, 2, 512] moving, 2 MACs
per cell per cycle) so the PE keeps up with the DMA stream even while the
power manager caps it at half clock during heavy DMA.  One-hot stationaries
are generated on the otherwise idle DVE from an int8 iota row and packed
targets (prepended to x chunk 0, bit-cast in SBUF) via a broadcast-AP
is_equal.  The stream is spread over all three DMA-capable queues
(sync/scalar HWDGE + gpsimd SWDGE).  The segment sums return as bf16; the
host forms the three Grams and does the count scaling + sqrt/relu/weighted
mean (<0.1% of the FLOPs) in float64.
"""

import numpy as np
import ml_dtypes

import concourse.bass as bass
import concourse.mybir as mybir
from concourse.bass_utils import run_bass_kernel_spmd

N = 4096
D = 2048
C = 128
MARGIN = 0.5
NCORES = 8
DCHUNK = D // NCORES          # 256 feature columns per core
P = 128                       # partitions / sample-block size
NB = N // P                   # 32 sample blocks
BWB = 512                     # fp8 bytes per partition per block (x1 256 | x2 256)
CW = C + NB                   # consts bytes per partition (iota row + targets)
# Chunks (in 128-sample blocks; all even so DoubleRow pairs never straddle),
# round-robin over the sync / scalar / gpsimd queues.  Entries on one queue
# stream back-to-back; each entry's semaphore lags its last byte by ~1.4us,
# so chunks are kept moderate for smooth PE gating, and the consts ride in
# front of chunk 0 (no separate tiny transfer).
CHUNKS = [2, 4, 4, 6, 6, 4, 4, 2]
QUEUE = ["sync", "scalar", "gpsimd", "sync", "scalar", "gpsimd", "sync", "scalar"]
NCHUNK = len(CHUNKS)
CHUNK_OFF = [sum(CHUNKS[:i]) for i in range(NCHUNK + 1)]
NWARM = 5                     # junk matmuls to ramp the PE clock

F32 = mybir.dt.float32
BF16 = mybir.dt.bfloat16
I8 = mybir.dt.int8
FP8 = mybir.dt.float8e4
NPBF16 = ml_dtypes.bfloat16
NPFP8 = ml_dtypes.float8_e4m3

_PROGRAM = None


def _build_program() -> bass.Bass:
    nc = bass.Bass()

    # x[:, 0:160]   = consts (int8 bits in the fp8 container): iota row
    #                 (iota[p, c] = c) then targets tgt[p, b] = targets[b*128+p].
    # x[:, 160 + b*512 : 160 + (b+1)*512] = fp8 [x1 | x2] for sample block b.
    x_in = nc.declare_dram_parameter("x", [P, CW + NB * BWB], FP8, isOutput=False)
    # sums[:, 0:256] = R segment sums for this core's d-chunk, [:, 256:512] = T
    sums_out = nc.declare_dram_parameter("sums", [P, 512], BF16, isOutput=True)

    import contextlib

    with contextlib.ExitStack() as stack:
        x_t = stack.enter_context(nc.sbuf_tensor([P, CW + NB * BWB], FP8))
        oh_t = stack.enter_context(nc.sbuf_tensor([P, NB * C], FP8))
        warm_t = stack.enter_context(nc.sbuf_tensor([P, 640], FP8))
        out_t = stack.enter_context(nc.sbuf_tensor([P, 512], BF16))
        psum_acc = stack.enter_context(nc.psum_tensor([P, 512], F32))
        psum_warm = stack.enter_context(nc.psum_tensor([P, 512], F32))

        def sem(name):
            return stack.enter_context(nc.semaphore(name))

        oh_gen = sem("oh_gen")
        x_sems = [sem(f"x_dma_{j}") for j in range(NCHUNK)]
        pe_done = sem("pe_done")
        vec_done = sem("vec_done")
        gp_done = sem("gp_done")
        dma_out = sem("dma_out")

        # Raw-bass semaphores are NOT cleared by the framework preamble; zero
        # them on gpsimd before the Block.  The Block-entry rendezvous keeps
        # every other engine from touching them until the clear has run.
        all_sems = [oh_gen] + x_sems + [pe_done, vec_done, gp_done, dma_out]
        nums = sorted(h.num for h in all_sems)
        assert nums == list(range(nums[0], nums[0] + len(nums))), nums
        nc.gpsimd.sem_clear(range(nums[0], nums[-1] + 1))

        def x_slice(j):
            a = CHUNK_OFF[j] * BWB + (0 if j == 0 else CW)
            b = CHUNK_OFF[j + 1] * BWB + CW
            return slice(a, b)

        def issue_chunks(eng, name):
            for j in range(NCHUNK):
                if QUEUE[j] != name:
                    continue
                sl = x_slice(j)
                eng.dma_start(out=x_t[:, sl], in_=x_in[:, sl]).then_inc(
                    x_sems[j], 16
                )

        with nc.Block(no_gpsimd_drain=True) as block:

            @block.sync
            def _(sync: bass.BassEngine):
                issue_chunks(sync, "sync")
                sync.wait_ge(vec_done, 1)
                sync.dma_start(
                    out=sums_out[:, 0:256], in_=out_t[:, 0:256]
                ).then_inc(dma_out, 16)
                sync.wait_ge(dma_out, 32)

            @block.scalar
            def _(scalar: bass.BassEngine):
                issue_chunks(scalar, "scalar")
                scalar.wait_ge(vec_done, 1)
                scalar.dma_start(
                    out=sums_out[:, 256:512], in_=out_t[:, 256:512]
                ).then_inc(dma_out, 16)
                scalar.wait_ge(dma_out, 32)

            @block.gpsimd
            def _(gp: bass.BassEngine):
                issue_chunks(gp, "gpsimd")

            @block.tensor
            def _(tensor: bass.BassEngine):
                # Ramp the PE clock (HAM gate) with junk matmuls while the
                # first DMA chunks are in flight.
                for _ in range(NWARM):
                    nc.tensor.matmul(
                        psum_warm[:],
                        warm_t[:, 0:128],
                        warm_t[:, 128:640],
                        start=True,
                        stop=True,
                    )
                for j in range(NCHUNK):
                    tensor.wait_ge(oh_gen, j + 1)
                    tensor.wait_ge(x_sems[j], 16)
                    for pr in range(CHUNK_OFF[j] // 2, CHUNK_OFF[j + 1] // 2):
                        lhsT = oh_t[:, pr * 2 * C : (pr + 1) * 2 * C].rearrange(
                            "p (r c) -> p r c", r=2
                        )
                        rhs = x_t[
                            :, CW + pr * 2 * BWB : CW + (pr + 1) * 2 * BWB
                        ].rearrange("p (r w) -> p r w", r=2)
                        nc.tensor.matmul(
                            psum_acc[:],
                            lhsT,
                            rhs,
                            start=(pr == 0),
                            stop=(pr == NB // 2 - 1),
                            perf_mode=mybir.MatmulPerfMode.DoubleRow,
                        )
                tensor.drain().then_inc(pe_done, 1)

            @block.vector
            def _(vector: bass.BassEngine):
                # One-hot generation: oh[p, b*C + c] = (c == targets[b*128+p])
                # as fp8, one tensor_tensor per x-chunk via broadcast APs.
                # consts ride in front of chunk 0; view them as int8.
                vector.wait_ge(x_sems[0], 16)
                consts_ap = x_t[:, 0:CW].bitcast(I8)
                iota_ap = consts_ap[:, 0:C].unsqueeze(1)
                for j in range(NCHUNK):
                    b0, b1 = CHUNK_OFF[j], CHUNK_OFF[j + 1]
                    nb = b1 - b0
                    o3 = oh_t[:, b0 * C : b1 * C].rearrange("p (j c) -> p j c", j=nb)
                    io3 = iota_ap.broadcast_to([P, nb, C])
                    tg3 = (
                        consts_ap[:, C + b0 : C + b1]
                        .unsqueeze(2)
                        .broadcast_to([P, nb, C])
                    )
                    nc.vector.tensor_tensor(
                        o3, io3, tg3, mybir.AluOpType.is_equal
                    ).then_inc(oh_gen, 1)
                vector.wait_ge(pe_done, 1)
                nc.vector.tensor_copy(out_t[:], psum_acc[:]).then_inc(vec_done, 1)

    return nc


def _get_program() -> bass.Bass:
    global _PROGRAM
    if _PROGRAM is None:
        _PROGRAM = _build_program()
    return _PROGRAM


def _make_in_maps(modal1, modal2, targets):
    x1 = np.asarray(modal1, dtype=np.float32)
    x2 = np.asarray(modal2, dtype=np.float32)
    targets = np.asarray(targets)

    x1q = x1.astype(NPFP8)
    x2q = x2.astype(NPFP8)

    tgt_pb = targets.reshape(NB, P).T.astype(np.int8)  # [p, b] = targets[b*128+p]
    iota = np.tile(np.arange(C, dtype=np.int8), (P, 1))
    consts = np.concatenate([iota, tgt_pb], axis=1).view(NPFP8)  # int8 bits

    in_maps = []
    for k in range(NCORES):
        sl = slice(k * DCHUNK, (k + 1) * DCHUNK)
        # [NB, P, 512] -> [P, NB*512]: block b cols = [x1 d-chunk | x2 d-chunk]
        blk = np.concatenate(
            [x1q[:, sl].reshape(NB, P, DCHUNK), x2q[:, sl].reshape(NB, P, DCHUNK)],
            axis=2,
        )
        xpk = blk.transpose(1, 0, 2).reshape(P, NB * BWB)
        in_maps.append(
            {"x": np.ascontiguousarray(np.concatenate([consts, xpk], axis=1))}
        )
    return in_maps


def _finish_on_host(sums_list, targets):
    """Recombine per-core segment sums, form class Grams, do class-pair loss."""
    P1 = np.zeros((C, C), np.float64)
    P2 = np.zeros((C, C), np.float64)
    P3 = np.zeros((C, C), np.float64)
    for s in sums_list:
        s = np.asarray(s, np.float64)
        R = s[:, 0:256]                      # [class, d-chunk]
        T = s[:, 256:512]
        P1 += R @ R.T
        P2 += T @ T.T
        P3 += R @ T.T

    n = np.bincount(targets, minlength=C).astype(np.float64)
    u = 1.0 / np.maximum(n, 1.0)

    S_CC = P1 + P2 + P3 + P3.T  # (R+T)(R+T)^T
    uu = np.outer(u, u)
    A1 = 0.5 * uu * (P1 + P3)    # meanR . ctr
    A2 = 0.5 * uu * (P2 + P3.T)  # meanT . ctr
    nR = u * u * np.diag(P1)
    nT = u * u * np.diag(P2)
    nCtr = 0.25 * u * u * np.diag(S_CC)

    W = np.outer(n, n)
    eye = np.eye(C)
    total = 0.0
    for A, nrm in ((A1, nR), (A2, nT)):
        sq = np.maximum(nrm[:, None] + nCtr[None, :] - 2.0 * A, 1e-12)
        d = np.sqrt(sq)
        dd = np.sqrt(d + 1e-10)
        term = eye * sq + (1.0 - eye) * np.maximum(MARGIN - dd, 0.0) ** 2
        total += (W * term).sum() / (float(N) * float(N))
    return np.asarray(total, dtype=np.float32)


def kernel(modal1_inputs, modal2_inputs, targets):
    nc = _get_program()
    in_maps = _make_in_maps(modal1_inputs, modal2_inputs, targets)
    res = run_bass_kernel_spmd(nc, in_maps, list(range(NCORES)))
    sums_list = [res.results[k]["sums"] for k in range(NCORES)]
    return _finish_on_host(sums_list, np.asarray(targets))
